# revision 1
# baseline (speedup 1.0000x reference)
"""BiLSTM-CRF NLL loss kernel for 8 Trainium2 NeuronCores.

Data-parallel over batch (128 samples/core). The partition function runs
as a linear-domain recurrence
    p_t = (M^T p_{t-1}) * exp(feats_t - dc_t)
with a host-computed per-step scalar normalizer schedule dc_t, in a
[32 tags x 128 samples] layout: PE does 2 small bf16 matmuls per step
(stationary 32x32 exp(transitions)), DVE one PSUM*SBUF multiply per step
per 64-sample chain; the two chains are phase-staggered so PE/DVE
round-trip latency overlaps. p_t history streams to DRAM in
partition-contiguous blocks; the host does the length-indexed readout
log(sum_j e^{trans[j,STOP]} p_t*[j]) + C_t in fp64.

The host pre-computes ef = exp(feats - dc) in bf16 and uploads it in the
on-chip layout (descriptor-friendly; every DMA is one fat contiguous run
per partition). Gold-score emissions are gathered on GPSIMD via
indirect_copy (indices shared per 16-partition group; a host-built mask
kills off-target rows) and reduced with partition_all_reduce, so the gold
path never touches PE/DVE; transition terms of the gold score are pure
(tags, transitions) index math on the host in fp64.
"""
import numpy as np
import ml_dtypes

B, L, T = 1024, 512, 32
START, STOP = 30, 31
NCORES = 8
BS = B // NCORES          # 128 samples per core
NBLK = 4                  # l-blocks
LB = L // NBLK            # 128 timesteps per block
CH = BS // 2              # 64 samples per chain

_PROG = None

TRACE = False
LAST_EXEC_NS = None


def _build_program():
    import concourse.bacc as bacc
    import concourse.mybir as mybir
    import concourse.tile as tile
    import concourse.bass_isa as bass_isa
    from concourse.tile_rust import add_dep_helper

    F32 = mybir.dt.float32
    BF16 = mybir.dt.bfloat16
    U16 = mybir.dt.uint16
    MULT = mybir.AluOpType.mult

    nc = bacc.Bacc("TRN2", target_bir_lowering=False, debug=False)

    # aef[32k+j, b*128+l_in] = exp(feats[b, 128k+l_in, j] - dc[128k+l_in]) bf16
    aef = nc.dram_tensor("aef", [128, BS * LB], BF16, kind="ExternalInput").ap()
    # ast[k, l_in, b*32+j] = feats[b, 128k+l_in, j] bf16 (gold-emission staging)
    ast = nc.dram_tensor("ast", [NBLK, LB, BS * T], BF16, kind="ExternalInput").ap()
    m32 = nc.dram_tensor("m32", [32, 32], BF16, kind="ExternalInput").ap()
    estart = nc.dram_tensor("estart", [32, 1], F32, kind="ExternalInput").ap()
    eidx = nc.dram_tensor("eidx", [128, NBLK * 128], U16, kind="ExternalInput").ap()
    emask = nc.dram_tensor("emask", [128, NBLK * 2048], BF16, kind="ExternalInput").ap()
    # hist[k, j, t_in*128 + b] = p_t[j, b] at t = 128k + t_in
    hist = nc.dram_tensor("hist", [NBLK, 32, LB * BS], BF16, kind="ExternalOutput").ap()
    emito = nc.dram_tensor("emito", [NBLK, 2048], F32, kind="ExternalOutput").ap()

    with tile.TileContext(nc) as tc:
        with (
            tc.tile_pool(name="consts", bufs=1) as consts,
            tc.tile_pool(name="efpool", bufs=1) as efpool,
            tc.tile_pool(name="stage", bufs=2) as stage,
            tc.tile_pool(name="goldp", bufs=2) as goldp,
            tc.tile_pool(name="ringp", bufs=2) as ringp,
            tc.tile_pool(name="upool", bufs=2, space="PSUM") as upool,
        ):
            m32_sb = consts.tile([32, 32], BF16)
            estart_sb = consts.tile([32, 1], F32)
            eidx_sb = consts.tile([128, NBLK * 128], U16)
            emask_sb = consts.tile([128, NBLK * 2048], BF16)
            nc.sync.dma_start(m32_sb[:], m32[:])
            nc.sync.dma_start(estart_sb[:], estart[:])
            nc.sync.dma_start(eidx_sb[:], eidx[:])
            nc.sync.dma_start(emask_sb[:], emask[:])

            ef_sb = efpool.tile([128, BS * LB], BF16)
            for k in range(NBLK):
                nc.sync.dma_start(ef_sb[32 * k:32 * (k + 1), :],
                                  aef[32 * k:32 * (k + 1), :])

            # ---------- gold emissions (GPSIMD only) ----------
            for k in range(NBLK):
                st = stage.tile([128, BS * T], BF16, name=f"st{k}", tag="st")
                nc.sync.dma_start(st[:], ast[k])
                gath = goldp.tile([128, 2048], BF16, name=f"gath{k}", tag="gath")
                # ISA limit: indirect_copy dst <= 1024 elems/partition
                for hh in range(2):
                    nc.gpsimd.indirect_copy(
                        gath[:, hh * 1024:(hh + 1) * 1024],
                        st[:],
                        eidx_sb[:, k * 128 + hh * 64:k * 128 + (hh + 1) * 64],
                        True,
                    )
                gm = goldp.tile([128, 2048], BF16, name=f"gm{k}", tag="gm")
                nc.gpsimd.tensor_tensor(
                    gm[:], gath[:], emask_sb[:, k * 2048:(k + 1) * 2048], MULT
                )
                par = goldp.tile([128, 2048], F32, name=f"par{k}", tag="par")
                nc.gpsimd.partition_all_reduce(
                    par[:], gm[:], channels=128, reduce_op=bass_isa.ReduceOp.add
                )
                nc.sync.dma_start(emito[k:k + 1, :], par[0:1, :])

            # ---------- recurrence ----------
            efv = ef_sb.rearrange("p (b l) -> p b l", l=LB)

            def ef_ap(t, h):
                k, l_in = divmod(t, LB)
                return efv[32 * k:32 * (k + 1), h * CH:(h + 1) * CH, l_in]

            prev = [None, None]
            stagger_from = None
            for k in range(NBLK):
                ring = ringp.tile([32, LB * BS], BF16, name=f"ring{k}", tag="ring")
                for t_in in range(LB):
                    t = k * LB + t_in
                    for h in range(2):
                        out_ap = ring[:, t_in * BS + h * CH:t_in * BS + (h + 1) * CH]
                        if t == 0:
                            ini = nc.vector.tensor_scalar(
                                out_ap, ef_ap(0, h), estart_sb[:, 0:1], None, MULT
                            )
                            if h == 1 and stagger_from is not None:
                                add_dep_helper(stagger_from.ins, ini.ins,
                                               sync=True, reason="phase stagger")
                        else:
                            u = upool.tile([32, CH], mybir.dt.float32,
                                           name=f"u{h}", tag=f"u{h}")
                            mm = nc.tensor.matmul(
                                u[:], m32_sb[:], prev[h], start=True, stop=True
                            )
                            if t == 1 and h == 0:
                                stagger_from = mm
                            nc.vector.tensor_tensor(out_ap, u[:], ef_ap(t, h), MULT)
                        prev[h] = out_ap
                nc.sync.dma_start(hist[k], ring[:])

    nc.compile()
    return nc


def _host_schedule(feats, transitions):
    """Per-step normalizer schedule C[l] from a 32-sample fp64 sub-simulation."""
    idx = np.linspace(0, feats.shape[0] - 1, 32).astype(np.int64)
    f = feats[idx].astype(np.float64)  # (32, L, T)
    tr = transitions.astype(np.float64)
    C = np.empty(L, np.float64)
    alpha = tr[START][None, :] + f[:, 0]
    C[0] = alpha.max(1).mean()
    eM = np.exp(tr)
    for l in range(1, L):
        m = alpha.max(1, keepdims=True)
        alpha = m + np.log(np.exp(alpha - m) @ eM) + f[:, l]
        C[l] = alpha.max(1).mean()
    return C


def _run(nc, in_maps):
    global LAST_EXEC_NS
    import os
    if os.environ.get("KERNEL_SIM"):
        from types import SimpleNamespace
        from concourse.bass_interp import CoreSim
        outs = []
        ncores = int(os.environ.get("KERNEL_SIM_CORES", str(NCORES)))
        for im in in_maps[:ncores]:
            sim = CoreSim(nc, require_finite=False, require_nnan=False)
            for k, v in im.items():
                sim.tensor(k)[:] = v
            sim.simulate()
            outs.append({n: np.array(sim.tensor(n)) for n in ("hist", "emito")})
        return SimpleNamespace(results=outs, exec_time_ns=None)
    from concourse.bass_utils import run_bass_kernel_spmd
    res = run_bass_kernel_spmd(nc, in_maps, list(range(NCORES)), trace=TRACE)
    LAST_EXEC_NS = res.exec_time_ns
    return res


def kernel(feats, transitions, tags, word_seq_lens):
    global _PROG

    feats = np.asarray(feats, np.float32)
    transitions = np.asarray(transitions, np.float32)
    tags = np.asarray(tags)
    lens = np.asarray(word_seq_lens).astype(np.int64)

    if _PROG is None:
        _PROG = _build_program()
    nc = _PROG

    # ---------------- host-side prep ----------------
    C = _host_schedule(feats, transitions)
    dC = np.diff(C, prepend=0.0)

    trf = transitions.astype(np.float64)
    m32 = np.exp(trf).astype(ml_dtypes.bfloat16)
    estart = np.ascontiguousarray(np.exp(trf[START]).astype(np.float32)[:, None])

    tags64 = tags.astype(np.int64)
    base_mask = (np.arange(L)[None, :] == 0) | (tags64 != 0)  # (B, L)

    in_maps = []
    for core in range(NCORES):
        sl = slice(core * BS, (core + 1) * BS)
        x = feats[sl]                                 # (BS, L, T)
        ex = np.exp(x - dC[None, :, None].astype(np.float32))
        # aef[32k+j, b*128+l_in] = ex[b, 128k+l_in, j]
        aef = np.ascontiguousarray(
            ex.reshape(BS, NBLK, LB, T).transpose(1, 3, 0, 2)
            .reshape(128, BS * LB).astype(ml_dtypes.bfloat16)
        )
        # ast[k, l_in, b*32+j] = x[b, 128k+l_in, j]
        ast = np.ascontiguousarray(
            x.reshape(BS, NBLK, LB, T).transpose(1, 2, 0, 3)
            .reshape(NBLK, LB, BS * T).astype(ml_dtypes.bfloat16)
        )
        tsh = tags64[sl]
        msh = base_mask[sl]
        eidx = np.empty((128, NBLK * 128), np.uint16)
        emask = np.zeros((128, NBLK, BS, 16), np.float32)
        for k in range(NBLK):
            lg = k * LB + np.arange(LB)
            eidx[:, k * 128:(k + 1) * 128] = (
                np.arange(BS)[None, :] * T + tsh[:, lg].T
            ).astype(np.uint16)
            emask[np.arange(LB), k, :, np.arange(LB) % 16] = \
                msh[:, lg].astype(np.float32).T
        emask = emask.reshape(128, NBLK * 2048).astype(ml_dtypes.bfloat16)
        in_maps.append({
            "aef": aef,
            "ast": ast,
            "m32": m32,
            "estart": estart,
            "eidx": eidx,
            "emask": np.ascontiguousarray(emask),
        })

    res = _run(nc, in_maps)
    results = res.results
    ncores_avail = len(results)

    # ---------------- host-side readout (fp64) ----------------
    estop = np.exp(trf[:, STOP])  # (T,)
    total_fwd = 0.0
    total_emit = 0.0
    for core in range(ncores_avail):
        r = results[core]
        h = np.asarray(r["hist"]).astype(np.float64)   # (NBLK, 32, LB*BS)
        em = np.asarray(r["emito"]).astype(np.float64)  # (NBLK, 2048)
        lsh = lens[core * BS:(core + 1) * BS]
        tstar = lsh - 1
        kk, tt = np.divmod(tstar, LB)
        pv = h[kk, :, tt * BS + np.arange(BS)]          # (BS, 32)
        total_fwd += (np.log(pv @ estop) + C[tstar]).sum()
        total_emit += em.reshape(NBLK, BS, 16).sum(axis=(0, 2)).sum()

    # gold transition terms on host
    tg = tags64
    mid_mask = (tg[:, 1:] != 0)
    trans_mid = (trf[tg[:, :-1], tg[:, 1:]] * mid_mask).sum()
    begin = trf[START, tg[:, 0]].sum()
    end_tag = np.take_along_axis(tg, (lens - 1)[:, None], axis=1)[:, 0]
    end = trf[end_tag, STOP].sum()
    total_gold = total_emit + trans_mid + begin + end

    return np.asarray(total_fwd - total_gold, np.float32)



# revision 2
# speedup vs baseline: 1.8713x; 1.8713x over previous
"""BiLSTM-CRF NLL loss kernel for 8 Trainium2 NeuronCores.

Data-parallel over batch (128 samples/core). The partition function runs
as a linear-domain recurrence
    p_t = (M^T p_{t-1}) * exp(feats_t - dc_t)
with a host-computed per-step scalar normalizer schedule dc_t.

On-chip layout: the 128x128 PE array holds a block-diagonal stationary
diag(expM, expM, expM, expM) loaded ONCE (redundant per-matmul LDWEIGHTS
are stripped post-legalization), so one matmul advances 4 groups of 16
samples: moving [128, 16] where partition 32g+j carries tag j of sample
group g. The 128 samples split into two phase-staggered chains of 64;
each step per chain is one matmul (PE) + one elementwise multiply
(chain A on DVE, chain B on GPSIMD) reading the PSUM result against the
resident exp(feats-dc) table and writing the p_t ring in SBUF. The ring
streams to DRAM per 128-step block; the host does the length-indexed
readout log(p_{len-1} . exp(trans[:,STOP])) + C_{len-1} in fp64.

The gold (labeled-path) score is pure index math on (feats, tags,
transitions) - O(B*L), a thousandth of the forward work - and is done
on the host in fp64.
"""
import os
import numpy as np
import ml_dtypes

B, L, T = 1024, 512, 32
START, STOP = 30, 31
NCORES = 8
BS = B // NCORES          # 128 samples per core
NBLK = 4                  # l-blocks
LB = L // NBLK            # 128 timesteps per block
G = 4                     # tag-blocks on partitions (block-diagonal)
F = 16                    # samples per group column
CH = G * F                # 64 samples per chain
NCH = BS // CH            # 2 chains

# engine for chain B's per-step multiply: "gpsimd" or "vector"
TT_B = os.environ.get("KERNEL_TT_B", "gpsimd")

_PROG = None
TRACE = False
LAST_EXEC_NS = None


def _strip_redundant_ldweights():
    """Patch tile_legalize so only the first of a run of identical
    InstLdweights per block survives: the stationary never changes, so
    the PE keeps it resident across all matmuls."""
    import concourse.tile as tile_mod

    orig = tile_mod.tile_legalize
    if getattr(orig, "_ldw_strip", False):
        return orig

    def patched(ordered, nc_):
        out = orig(ordered, nc_)
        for bb in list(out.keys()):
            insts = out[bb]
            kept = []
            prev_sig = None
            for inst in insts:
                if type(inst).__name__ == "InstLdweights":
                    sig = str(inst.ins)
                    if sig == prev_sig:
                        continue
                    prev_sig = sig
                kept.append(inst)
            out[bb] = kept
        return out

    patched._ldw_strip = True
    tile_mod.tile_legalize = patched
    return orig


def _build_program():
    import concourse.bacc as bacc
    import concourse.mybir as mybir
    import concourse.tile as tile

    F32 = mybir.dt.float32
    BF16 = mybir.dt.bfloat16
    MULT = mybir.AluOpType.mult

    nc = bacc.Bacc("TRN2", target_bir_lowering=False, debug=False)

    # aef[c][32g+j, t*16+s] = exp(feats[b, t, j] - dc[t]) bf16,
    #   b = core_base + 64c + 16g + s
    aef = [nc.dram_tensor(f"aef{c}", [128, L * F], BF16, kind="ExternalInput").ap()
           for c in range(NCH)]
    bd4 = nc.dram_tensor("bd4", [128, 128], BF16, kind="ExternalInput").ap()
    estart = nc.dram_tensor("estart", [128, 1], F32, kind="ExternalInput").ap()
    # hist[c][k, 32g+j, t_in*16+s] = p_t[32g+j, s] at t = 128k + t_in
    hist = [nc.dram_tensor(f"hist{c}", [NBLK, 128, LB * F], BF16,
                           kind="ExternalOutput").ap()
            for c in range(NCH)]

    restore = _strip_redundant_ldweights()
    try:
        with tile.TileContext(nc) as tc:
            with (
                tc.tile_pool(name="consts", bufs=1) as consts,
                tc.tile_pool(name="efpool", bufs=1) as efpool,
                tc.tile_pool(name="ringp", bufs=2) as ringp,
                tc.tile_pool(name="upool", bufs=2, space="PSUM") as upool,
            ):
                bd4_sb = consts.tile([128, 128], BF16)
                estart_sb = consts.tile([128, 1], F32)
                nc.sync.dma_start(bd4_sb[:], bd4[:])
                nc.sync.dma_start(estart_sb[:], estart[:])

                ef_sb = [[efpool.tile([128, LB * F], BF16, name=f"ef{c}_{k}")
                          for k in range(NBLK)] for c in range(NCH)]
                for c in range(NCH):
                    for k in range(NBLK):
                        nc.sync.dma_start(
                            ef_sb[c][k][:],
                            aef[c][:, k * LB * F:(k + 1) * LB * F])

                tt_eng = [nc.vector,
                          nc.gpsimd if TT_B == "gpsimd" else nc.vector]

                prev = [None] * NCH
                for k in range(NBLK):
                    rings = [ringp.tile([128, LB * F], BF16,
                                        name=f"ring{c}_{k}", tag=f"ring{c}")
                             for c in range(NCH)]
                    for t_in in range(LB):
                        t = k * LB + t_in
                        for c in range(NCH):
                            out_ap = rings[c][:, t_in * F:(t_in + 1) * F]
                            ef_ap = ef_sb[c][k][:, t_in * F:(t_in + 1) * F]
                            if t == 0:
                                tt_eng[c].tensor_scalar(
                                    out_ap, ef_ap, estart_sb[:, 0:1],
                                    None, MULT)
                            else:
                                u = upool.tile([128, F], F32,
                                               name=f"u{c}", tag=f"u{c}")
                                nc.tensor.matmul(u[:], bd4_sb[:], prev[c],
                                                 start=True, stop=True)
                                tt_eng[c].tensor_tensor(
                                    out_ap, u[:], ef_ap, MULT)
                            prev[c] = out_ap
                    for c in range(NCH):
                        nc.sync.dma_start(hist[c][k], rings[c][:])
        nc.compile()
    finally:
        import concourse.tile as tile_mod
        tile_mod.tile_legalize = restore

    return nc


def _host_schedule(feats, transitions):
    """Per-step normalizer schedule C[l] from a 32-sample fp64 sub-simulation."""
    idx = np.linspace(0, feats.shape[0] - 1, 32).astype(np.int64)
    f = feats[idx].astype(np.float64)  # (32, L, T)
    tr = transitions.astype(np.float64)
    C = np.empty(L, np.float64)
    alpha = tr[START][None, :] + f[:, 0]
    C[0] = alpha.max(1).mean()
    eM = np.exp(tr)
    for l in range(1, L):
        m = alpha.max(1, keepdims=True)
        alpha = m + np.log(np.exp(alpha - m) @ eM) + f[:, l]
        C[l] = alpha.max(1).mean()
    return C


def _run(nc, in_maps):
    global LAST_EXEC_NS
    if os.environ.get("KERNEL_SIM"):
        from types import SimpleNamespace
        from concourse.bass_interp import CoreSim
        outs = []
        ncores = int(os.environ.get("KERNEL_SIM_CORES", str(NCORES)))
        for im in in_maps[:ncores]:
            sim = CoreSim(nc, require_finite=False, require_nnan=False)
            for k, v in im.items():
                sim.tensor(k)[:] = v
            sim.simulate()
            outs.append({n: np.array(sim.tensor(n))
                         for n in ("hist0", "hist1")})
        return SimpleNamespace(results=outs, exec_time_ns=None)
    from concourse.bass_utils import run_bass_kernel_spmd
    res = run_bass_kernel_spmd(nc, in_maps, list(range(NCORES)), trace=TRACE)
    LAST_EXEC_NS = res.exec_time_ns
    return res


def kernel(feats, transitions, tags, word_seq_lens):
    global _PROG

    feats = np.asarray(feats, np.float32)
    transitions = np.asarray(transitions, np.float32)
    tags64 = np.asarray(tags).astype(np.int64)
    lens = np.asarray(word_seq_lens).astype(np.int64)

    if _PROG is None:
        _PROG = _build_program()
    nc = _PROG

    # ---------------- host-side prep ----------------
    C = _host_schedule(feats, transitions)
    dC = np.diff(C, prepend=0.0)

    trf = transitions.astype(np.float64)
    eM = np.exp(trf)
    bd4 = np.zeros((128, 128), np.float64)
    for g in range(G):
        bd4[32 * g:32 * (g + 1), 32 * g:32 * (g + 1)] = eM
    bd4 = bd4.astype(ml_dtypes.bfloat16)
    estart = np.tile(eM[START], G).astype(np.float32)[:, None]
    estart = np.ascontiguousarray(estart)

    in_maps = []
    for core in range(NCORES):
        sl = slice(core * BS, (core + 1) * BS)
        x = feats[sl]                                 # (BS, L, T)
        ex = np.exp(x - dC[None, :, None].astype(np.float32))
        im = {"bd4": bd4, "estart": estart}
        for c in range(NCH):
            # [64 samples, L, T] -> [G, F, L, T] -> [G, T, L, F] -> [128, L*F]
            chunk = ex[c * CH:(c + 1) * CH]
            im[f"aef{c}"] = np.ascontiguousarray(
                chunk.reshape(G, F, L, T).transpose(0, 3, 2, 1)
                .reshape(128, L * F).astype(ml_dtypes.bfloat16))
        in_maps.append(im)

    res = _run(nc, in_maps)
    results = res.results
    ncores_avail = len(results)

    # ---------------- host-side readout (fp64) ----------------
    estop = np.exp(trf[:, STOP])  # (T,)
    total_fwd = 0.0
    for core in range(ncores_avail):
        r = results[core]
        lsh = lens[core * BS:(core + 1) * BS]
        tstar = lsh - 1                                  # (BS,)
        kk, tt = np.divmod(tstar, LB)
        for c in range(NCH):
            h = np.asarray(r[f"hist{c}"]).astype(np.float64)
            # sample r0 in [0, CH): group g = r0 // F, col s = r0 % F
            r0 = np.arange(CH)
            g = r0 // F
            s = r0 % F
            ks = kk[c * CH:(c + 1) * CH]
            ts = tt[c * CH:(c + 1) * CH]
            # p_vec[r0, j] = h[ks, 32*g+j, ts*F+s]
            pv = h[ks[:, None], (32 * g)[:, None] + np.arange(T)[None, :],
                   (ts * F + s)[:, None]]             # (CH, T)
            total_fwd += (np.log(pv @ estop)
                          + C[tstar[c * CH:(c + 1) * CH]]).sum()

    # ---------------- gold score on host (fp64) ----------------
    f64 = feats.astype(np.float64)
    emit = np.take_along_axis(f64, tags64[:, :, None], axis=2)[:, :, 0]  # (B,L)
    lmask = np.arange(L)[None, :] < lens[:, None]
    emit_sum = (emit * lmask).sum()
    mid_mask = (tags64[:, 1:] != 0)
    trans_mid = (trf[tags64[:, :-1], tags64[:, 1:]] * mid_mask).sum()
    begin = trf[START, tags64[:, 0]].sum()
    end_tag = np.take_along_axis(tags64, (lens - 1)[:, None], axis=1)[:, 0]
    end = trf[end_tag, STOP].sum()
    total_gold = emit_sum + trans_mid + begin + end

    return np.asarray(total_fwd - total_gold, np.float32)


# revision 8
# speedup vs baseline: 1.9478x; 1.0409x over previous
"""BiLSTM-CRF NLL loss kernel for 8 Trainium2 NeuronCores.

Data-parallel over batch (128 samples/core). The partition function runs
as a linear-domain recurrence
    p_t = (M^T p_{t-1}) * exp(feats_t - dc_t)
with a host-computed per-step scalar normalizer schedule dc_t.

On-chip layout: the 128x128 PE array holds a block-diagonal stationary
diag(expM, expM, expM, expM) loaded ONCE (redundant per-matmul LDWEIGHTS
are stripped post-legalization), so one matmul advances 4 groups of 16
samples: moving [128, 16] where partition 32g+j carries tag j of sample
group g. The 128 samples split into two phase-staggered chains of 64;
each step per chain is one matmul (PE) + one DVE tensor_tensor reading
the PSUM result against the resident exp(feats-dc) table and writing
the p_t ring in SBUF (GPSIMD cannot read PSUM; the steady state is
latency-bound at ~435ns/step = PE array latency + 2 semaphore hops +
DVE PSUM access). p_0 is folded into the ef table on the host. The
ring streams to DRAM per half-block; the host does the length-indexed
readout log(p_{len-1} . exp(trans[:,STOP])) + C_{len-1} in fp64.

The gold (labeled-path) score is pure index math on (feats, tags,
transitions) - O(B*L), a thousandth of the forward work - and is done
on the host in fp64.
"""
import os
import numpy as np
import ml_dtypes

B, L, T = 1024, 512, 32
START, STOP = 30, 31
NCORES = 8
BS = B // NCORES          # 128 samples per core
NBLK = 4                  # l-blocks
LB = L // NBLK            # 128 timesteps per block
G = 4                     # tag-blocks on partitions (block-diagonal)
F = 16                    # samples per group column
CH = G * F                # 64 samples per chain
NCH = BS // CH            # 2 chains

_PROG = None
TRACE = False
LAST_EXEC_NS = None


def _strip_redundant_ldweights():
    """Patch tile_legalize so only the first of a run of identical
    InstLdweights per block survives: the stationary never changes, so
    the PE keeps it resident across all matmuls."""
    import concourse.tile as tile_mod

    orig = tile_mod.tile_legalize
    if getattr(orig, "_ldw_strip", False):
        return orig

    def patched(ordered, nc_):
        out = orig(ordered, nc_)
        for bb in list(out.keys()):
            insts = out[bb]
            kept = []
            prev_sig = None
            for inst in insts:
                if type(inst).__name__ == "InstLdweights":
                    sig = str(inst.ins)
                    if sig == prev_sig:
                        continue
                    prev_sig = sig
                kept.append(inst)
            out[bb] = kept
        return out

    patched._ldw_strip = True
    tile_mod.tile_legalize = patched
    return orig


def _build_program():
    import concourse.bacc as bacc
    import concourse.mybir as mybir
    import concourse.tile as tile

    F32 = mybir.dt.float32
    BF16 = mybir.dt.bfloat16
    MULT = mybir.AluOpType.mult

    nc = bacc.Bacc("TRN2", target_bir_lowering=False, debug=False)

    # aef[c][32g+j, t*16+s] = exp(feats[b, t, j] - dc[t]) bf16,
    #   b = core_base + 64c + 16g + s; cols 0:16 pre-scaled by exp(tr[START,:])
    aef = [nc.dram_tensor(f"aef{c}", [128, L * F], BF16, kind="ExternalInput").ap()
           for c in range(NCH)]
    bd4 = nc.dram_tensor("bd4", [128, 128], BF16, kind="ExternalInput").ap()
    # hist[c][k, 32g+j, t_in*16+s] = p_t[32g+j, s] at t = 128k + t_in
    # (block 0 cols 0:16, i.e. t=0, are never written; the host handles
    #  len==1 samples directly)
    hist = [nc.dram_tensor(f"hist{c}", [NBLK, 128, LB * F], BF16,
                           kind="ExternalOutput").ap()
            for c in range(NCH)]

    HLF = LB * F // 2  # half-block ef/hist DMA granularity

    restore = _strip_redundant_ldweights()
    try:
        with tile.TileContext(nc) as tc:
            with (
                tc.tile_pool(name="consts", bufs=1) as consts,
                tc.tile_pool(name="efpool", bufs=1) as efpool,
                tc.tile_pool(name="ringp", bufs=2) as ringp,
                tc.tile_pool(name="upool", bufs=2, space="PSUM") as upool,
            ):
                bd4_sb = consts.tile([128, 128], BF16)
                nc.sync.dma_start(bd4_sb[:], bd4[:])

                ef_sb = [[efpool.tile([128, LB * F], BF16, name=f"ef{c}_{k}")
                          for k in range(NBLK)] for c in range(NCH)]
                # block-0 halves first (head gates the pipeline start)
                for h in range(2):
                    for c in range(NCH):
                        nc.sync.dma_start(
                            ef_sb[c][0][:, h * HLF:(h + 1) * HLF],
                            aef[c][:, h * HLF:(h + 1) * HLF])
                for k in range(1, NBLK):
                    for c in range(NCH):
                        nc.sync.dma_start(
                            ef_sb[c][k][:],
                            aef[c][:, k * LB * F:(k + 1) * LB * F])

                prev = [None] * NCH
                for k in range(NBLK):
                    rings = [ringp.tile([128, LB * F], BF16,
                                        name=f"ring{c}_{k}", tag=f"ring{c}")
                             for c in range(NCH)]
                    for t_in in range(LB):
                        t = k * LB + t_in
                        for c in range(NCH):
                            out_ap = rings[c][:, t_in * F:(t_in + 1) * F]
                            ef_ap = ef_sb[c][k][:, t_in * F:(t_in + 1) * F]
                            if t == 0:
                                # p_0 = exp(tr[START,:]) * ef_0 is baked into
                                # aef cols 0:16 on the host; copy into the
                                # ring so hist block 0 is fully defined
                                nc.vector.tensor_scalar(
                                    out_ap, ef_ap, 1.0, None, MULT)
                                prev[c] = ef_ap
                                continue
                            u = upool.tile([128, F], F32,
                                           name=f"u{c}", tag=f"u{c}")
                            nc.tensor.matmul(u[:], bd4_sb[:], prev[c],
                                             start=True, stop=True)
                            nc.vector.tensor_tensor(out_ap, u[:], ef_ap, MULT)
                            prev[c] = out_ap
                        if t_in == LB // 2 - 1:
                            for c in range(NCH):
                                nc.sync.dma_start(hist[c][k][:, 0:HLF],
                                                  rings[c][:, 0:HLF])
                    for c in range(NCH):
                        nc.sync.dma_start(hist[c][k][:, HLF:],
                                          rings[c][:, HLF:])
        nc.compile()
    finally:
        import concourse.tile as tile_mod
        tile_mod.tile_legalize = restore

    return nc


def _host_schedule(feats, transitions):
    """Per-step normalizer schedule C[l] from a 32-sample fp64 sub-simulation."""
    idx = np.linspace(0, feats.shape[0] - 1, 32).astype(np.int64)
    f = feats[idx].astype(np.float64)  # (32, L, T)
    tr = transitions.astype(np.float64)
    C = np.empty(L, np.float64)
    alpha = tr[START][None, :] + f[:, 0]
    C[0] = alpha.max(1).mean()
    eM = np.exp(tr)
    for l in range(1, L):
        m = alpha.max(1, keepdims=True)
        alpha = m + np.log(np.exp(alpha - m) @ eM) + f[:, l]
        C[l] = alpha.max(1).mean()
    return C


def _run(nc, in_maps):
    global LAST_EXEC_NS
    if os.environ.get("KERNEL_SIM"):
        from types import SimpleNamespace
        from concourse.bass_interp import CoreSim
        outs = []
        ncores = int(os.environ.get("KERNEL_SIM_CORES", str(NCORES)))
        for im in in_maps[:ncores]:
            sim = CoreSim(nc, require_finite=False, require_nnan=False)
            for k, v in im.items():
                sim.tensor(k)[:] = v
            sim.simulate()
            outs.append({n: np.array(sim.tensor(n))
                         for n in ("hist0", "hist1")})
        return SimpleNamespace(results=outs, exec_time_ns=None)
    from concourse.bass_utils import run_bass_kernel_spmd
    res = run_bass_kernel_spmd(nc, in_maps, list(range(NCORES)), trace=TRACE)
    LAST_EXEC_NS = res.exec_time_ns
    return res


def kernel(feats, transitions, tags, word_seq_lens):
    global _PROG

    feats = np.asarray(feats, np.float32)
    transitions = np.asarray(transitions, np.float32)
    tags64 = np.asarray(tags).astype(np.int64)
    lens = np.asarray(word_seq_lens).astype(np.int64)

    if _PROG is None:
        _PROG = _build_program()
    nc = _PROG

    # ---------------- host-side prep ----------------
    C = _host_schedule(feats, transitions)
    dC = np.diff(C, prepend=0.0)

    trf = transitions.astype(np.float64)
    eM = np.exp(trf)
    bd4 = np.zeros((128, 128), np.float64)
    for g in range(G):
        bd4[32 * g:32 * (g + 1), 32 * g:32 * (g + 1)] = eM
    bd4 = bd4.astype(ml_dtypes.bfloat16)

    in_maps = []
    for core in range(NCORES):
        sl = slice(core * BS, (core + 1) * BS)
        x = feats[sl]                                 # (BS, L, T)
        ex = np.exp(x - dC[None, :, None].astype(np.float32))
        ex[:, 0, :] *= eM[START][None, :].astype(np.float32)  # p_0 baked in
        im = {"bd4": bd4}
        for c in range(NCH):
            # [64 samples, L, T] -> [G, F, L, T] -> [G, T, L, F] -> [128, L*F]
            chunk = ex[c * CH:(c + 1) * CH]
            im[f"aef{c}"] = np.ascontiguousarray(
                chunk.reshape(G, F, L, T).transpose(0, 3, 2, 1)
                .reshape(128, L * F).astype(ml_dtypes.bfloat16))
        in_maps.append(im)

    res = _run(nc, in_maps)
    results = res.results
    ncores_avail = len(results)

    # ---------------- host-side readout (fp64) ----------------
    estop = np.exp(trf[:, STOP])  # (T,)
    total_fwd = 0.0
    for core in range(ncores_avail):
        r = results[core]
        lsh = lens[core * BS:(core + 1) * BS]
        tstar = lsh - 1                                  # (BS,)
        kk, tt = np.divmod(tstar, LB)
        for c in range(NCH):
            h = np.asarray(r[f"hist{c}"]).astype(np.float64)
            # sample r0 in [0, CH): group g = r0 // F, col s = r0 % F
            r0 = np.arange(CH)
            g = r0 // F
            s = r0 % F
            ks = kk[c * CH:(c + 1) * CH]
            ts = tt[c * CH:(c + 1) * CH]
            # p_vec[r0, j] = h[ks, 32*g+j, ts*F+s]
            pv = h[ks[:, None], (32 * g)[:, None] + np.arange(T)[None, :],
                   (ts * F + s)[:, None]]             # (CH, T)
            tsr = tstar[c * CH:(c + 1) * CH]
            val = np.log(pv @ estop) + C[tsr]
            if np.any(tsr == 0):
                # len==1: t*=0 was never computed on device; do it directly
                gb = np.where(tsr == 0)[0]
                bidx = core * BS + c * CH + gb
                a0 = (trf[START][None, :]
                      + feats[bidx, 0].astype(np.float64)
                      + trf[:, STOP][None, :])
                m = a0.max(1)
                val[gb] = m + np.log(np.exp(a0 - m[:, None]).sum(1))
            total_fwd += val.sum()

    # ---------------- gold score on host (fp64) ----------------
    f64 = feats.astype(np.float64)
    emit = np.take_along_axis(f64, tags64[:, :, None], axis=2)[:, :, 0]  # (B,L)
    lmask = np.arange(L)[None, :] < lens[:, None]
    emit_sum = (emit * lmask).sum()
    mid_mask = (tags64[:, 1:] != 0)
    trans_mid = (trf[tags64[:, :-1], tags64[:, 1:]] * mid_mask).sum()
    begin = trf[START, tags64[:, 0]].sum()
    end_tag = np.take_along_axis(tags64, (lens - 1)[:, None], axis=1)[:, 0]
    end = trf[end_tag, STOP].sum()
    total_gold = emit_sum + trans_mid + begin + end

    return np.asarray(total_fwd - total_gold, np.float32)


# revision 9
# speedup vs baseline: 1.9627x; 1.0077x over previous
"""BiLSTM-CRF NLL loss kernel for 8 Trainium2 NeuronCores.

Data-parallel over batch (128 samples/core). The partition function runs
as a linear-domain recurrence
    p_t = (M^T p_{t-1}) * exp(feats_t - dc_t)
with a host-computed per-step scalar normalizer schedule dc_t.

On-chip layout: the 128x128 PE array holds a block-diagonal stationary
diag(expM, expM, expM, expM) loaded ONCE (redundant per-matmul LDWEIGHTS
are stripped post-legalization), so one matmul advances 4 groups of 16
samples: moving [128, 16] where partition 32g+j carries tag j of sample
group g. The 128 samples split into two phase-staggered chains of 64;
each step per chain is one matmul (PE) + one DVE tensor_tensor reading
the PSUM result against the resident exp(feats-dc) table and writing
the p_t ring in SBUF (GPSIMD cannot read PSUM; the steady state is
latency-bound at ~435ns/step = PE array latency + 2 semaphore hops +
DVE PSUM access). p_0 is folded into the ef table on the host. The
ring streams to DRAM per half-block; the host does the length-indexed
readout log(p_{len-1} . exp(trans[:,STOP])) + C_{len-1} in fp64.

The gold (labeled-path) score is pure index math on (feats, tags,
transitions) - O(B*L), a thousandth of the forward work - and is done
on the host in fp64.
"""
import os
import numpy as np
import ml_dtypes

B, L, T = 1024, 512, 32
START, STOP = 30, 31
NCORES = 8
BS = B // NCORES          # 128 samples per core
NBLK = 4                  # l-blocks
LB = L // NBLK            # 128 timesteps per block
G = 4                     # tag-blocks on partitions (block-diagonal)
F = 16                    # samples per group column
CH = G * F                # 64 samples per chain
NCH = BS // CH            # 2 chains

_PROG = None
TRACE = False
LAST_EXEC_NS = None


def _strip_redundant_ldweights():
    """Patch tile_legalize so only the first of a run of identical
    InstLdweights per block survives: the stationary never changes, so
    the PE keeps it resident across all matmuls."""
    import concourse.tile as tile_mod

    orig = tile_mod.tile_legalize
    if getattr(orig, "_ldw_strip", False):
        return orig

    def patched(ordered, nc_):
        out = orig(ordered, nc_)
        for bb in list(out.keys()):
            insts = out[bb]
            kept = []
            prev_sig = None
            for inst in insts:
                if type(inst).__name__ == "InstLdweights":
                    sig = str(inst.ins)
                    if sig == prev_sig:
                        continue
                    prev_sig = sig
                kept.append(inst)
            out[bb] = kept
        return out

    patched._ldw_strip = True
    tile_mod.tile_legalize = patched
    return orig


def _build_program():
    import concourse.bacc as bacc
    import concourse.mybir as mybir
    import concourse.tile as tile

    F32 = mybir.dt.float32
    BF16 = mybir.dt.bfloat16
    MULT = mybir.AluOpType.mult

    nc = bacc.Bacc("TRN2", target_bir_lowering=False, debug=False)

    # aef[c][32g+j, t*16+s] = exp(feats[b, t, j] - dc[t]) bf16,
    #   b = core_base + 64c + 16g + s; cols 0:16 pre-scaled by exp(tr[START,:])
    aef = [nc.dram_tensor(f"aef{c}", [128, L * F], BF16, kind="ExternalInput").ap()
           for c in range(NCH)]
    bd4 = nc.dram_tensor("bd4", [128, 128], BF16, kind="ExternalInput").ap()
    # hist[c][k, 32g+j, t_in*16+s] = p_t[32g+j, s] at t = 128k + t_in
    # (block 0 cols 0:16, i.e. t=0, are never written; the host handles
    #  len==1 samples directly)
    hist = [nc.dram_tensor(f"hist{c}", [NBLK, 128, LB * F], BF16,
                           kind="ExternalOutput").ap()
            for c in range(NCH)]

    HLF = LB * F // 2  # half-block ef/hist DMA granularity

    restore = _strip_redundant_ldweights()
    try:
        with tile.TileContext(nc) as tc:
            with (
                tc.tile_pool(name="consts", bufs=1) as consts,
                tc.tile_pool(name="efpool", bufs=1) as efpool,
                tc.tile_pool(name="ringp", bufs=2) as ringp,
                tc.tile_pool(name="upool", bufs=2, space="PSUM") as upool,
            ):
                bd4_sb = consts.tile([128, 128], BF16)
                nc.sync.dma_start(bd4_sb[:], bd4[:])

                ef_sb = [[efpool.tile([128, LB * F], BF16, name=f"ef{c}_{k}")
                          for k in range(NBLK)] for c in range(NCH)]
                # block-0 in head-first chunks (the 16-step head gates the
                # pipeline start), then blocks 1-3 whole
                HEAD = 16 * F
                for lo, hi in ((0, HEAD), (HEAD, HLF), (HLF, 2 * HLF)):
                    for c in range(NCH):
                        nc.sync.dma_start(ef_sb[c][0][:, lo:hi],
                                          aef[c][:, lo:hi])
                for k in range(1, NBLK):
                    for c in range(NCH):
                        nc.sync.dma_start(
                            ef_sb[c][k][:],
                            aef[c][:, k * LB * F:(k + 1) * LB * F])

                prev = [None] * NCH
                for k in range(NBLK):
                    rings = [ringp.tile([128, LB * F], BF16,
                                        name=f"ring{c}_{k}", tag=f"ring{c}")
                             for c in range(NCH)]
                    for t_in in range(LB):
                        t = k * LB + t_in
                        for c in range(NCH):
                            out_ap = rings[c][:, t_in * F:(t_in + 1) * F]
                            ef_ap = ef_sb[c][k][:, t_in * F:(t_in + 1) * F]
                            if t == 0:
                                # p_0 = exp(tr[START,:]) * ef_0 is baked into
                                # aef cols 0:16 on the host; copy into the
                                # ring so hist block 0 is fully defined
                                nc.vector.tensor_scalar(
                                    out_ap, ef_ap, 1.0, None, MULT)
                                prev[c] = ef_ap
                                continue
                            u = upool.tile([128, F], F32,
                                           name=f"u{c}", tag=f"u{c}")
                            nc.tensor.matmul(u[:], bd4_sb[:], prev[c],
                                             start=True, stop=True)
                            nc.vector.tensor_tensor(out_ap, u[:], ef_ap, MULT)
                            prev[c] = out_ap
                        if t_in == LB // 2 - 1:
                            for c in range(NCH):
                                nc.sync.dma_start(hist[c][k][:, 0:HLF],
                                                  rings[c][:, 0:HLF])
                    for c in range(NCH):
                        nc.sync.dma_start(hist[c][k][:, HLF:],
                                          rings[c][:, HLF:])
        nc.compile()
    finally:
        import concourse.tile as tile_mod
        tile_mod.tile_legalize = restore

    return nc


def _host_schedule(feats, transitions):
    """Per-step normalizer schedule C[l] from a 32-sample fp64 sub-simulation."""
    idx = np.linspace(0, feats.shape[0] - 1, 32).astype(np.int64)
    f = feats[idx].astype(np.float64)  # (32, L, T)
    tr = transitions.astype(np.float64)
    C = np.empty(L, np.float64)
    alpha = tr[START][None, :] + f[:, 0]
    C[0] = alpha.max(1).mean()
    eM = np.exp(tr)
    for l in range(1, L):
        m = alpha.max(1, keepdims=True)
        alpha = m + np.log(np.exp(alpha - m) @ eM) + f[:, l]
        C[l] = alpha.max(1).mean()
    return C


def _run(nc, in_maps):
    global LAST_EXEC_NS
    if os.environ.get("KERNEL_SIM"):
        from types import SimpleNamespace
        from concourse.bass_interp import CoreSim
        outs = []
        ncores = int(os.environ.get("KERNEL_SIM_CORES", str(NCORES)))
        for im in in_maps[:ncores]:
            sim = CoreSim(nc, require_finite=False, require_nnan=False)
            for k, v in im.items():
                sim.tensor(k)[:] = v
            sim.simulate()
            outs.append({n: np.array(sim.tensor(n))
                         for n in ("hist0", "hist1")})
        return SimpleNamespace(results=outs, exec_time_ns=None)
    from concourse.bass_utils import run_bass_kernel_spmd
    res = run_bass_kernel_spmd(nc, in_maps, list(range(NCORES)), trace=TRACE)
    LAST_EXEC_NS = res.exec_time_ns
    return res


def kernel(feats, transitions, tags, word_seq_lens):
    global _PROG

    feats = np.asarray(feats, np.float32)
    transitions = np.asarray(transitions, np.float32)
    tags64 = np.asarray(tags).astype(np.int64)
    lens = np.asarray(word_seq_lens).astype(np.int64)

    if _PROG is None:
        _PROG = _build_program()
    nc = _PROG

    # ---------------- host-side prep ----------------
    C = _host_schedule(feats, transitions)
    dC = np.diff(C, prepend=0.0)

    trf = transitions.astype(np.float64)
    eM = np.exp(trf)
    bd4 = np.zeros((128, 128), np.float64)
    for g in range(G):
        bd4[32 * g:32 * (g + 1), 32 * g:32 * (g + 1)] = eM
    bd4 = bd4.astype(ml_dtypes.bfloat16)

    in_maps = []
    for core in range(NCORES):
        sl = slice(core * BS, (core + 1) * BS)
        x = feats[sl]                                 # (BS, L, T)
        ex = np.exp(x - dC[None, :, None].astype(np.float32))
        ex[:, 0, :] *= eM[START][None, :].astype(np.float32)  # p_0 baked in
        im = {"bd4": bd4}
        for c in range(NCH):
            # [64 samples, L, T] -> [G, F, L, T] -> [G, T, L, F] -> [128, L*F]
            chunk = ex[c * CH:(c + 1) * CH]
            im[f"aef{c}"] = np.ascontiguousarray(
                chunk.reshape(G, F, L, T).transpose(0, 3, 2, 1)
                .reshape(128, L * F).astype(ml_dtypes.bfloat16))
        in_maps.append(im)

    res = _run(nc, in_maps)
    results = res.results
    ncores_avail = len(results)

    # ---------------- host-side readout (fp64) ----------------
    estop = np.exp(trf[:, STOP])  # (T,)
    total_fwd = 0.0
    for core in range(ncores_avail):
        r = results[core]
        lsh = lens[core * BS:(core + 1) * BS]
        tstar = lsh - 1                                  # (BS,)
        kk, tt = np.divmod(tstar, LB)
        for c in range(NCH):
            h = np.asarray(r[f"hist{c}"]).astype(np.float64)
            # sample r0 in [0, CH): group g = r0 // F, col s = r0 % F
            r0 = np.arange(CH)
            g = r0 // F
            s = r0 % F
            ks = kk[c * CH:(c + 1) * CH]
            ts = tt[c * CH:(c + 1) * CH]
            # p_vec[r0, j] = h[ks, 32*g+j, ts*F+s]
            pv = h[ks[:, None], (32 * g)[:, None] + np.arange(T)[None, :],
                   (ts * F + s)[:, None]]             # (CH, T)
            tsr = tstar[c * CH:(c + 1) * CH]
            val = np.log(pv @ estop) + C[tsr]
            if np.any(tsr == 0):
                # len==1: t*=0 was never computed on device; do it directly
                gb = np.where(tsr == 0)[0]
                bidx = core * BS + c * CH + gb
                a0 = (trf[START][None, :]
                      + feats[bidx, 0].astype(np.float64)
                      + trf[:, STOP][None, :])
                m = a0.max(1)
                val[gb] = m + np.log(np.exp(a0 - m[:, None]).sum(1))
            total_fwd += val.sum()

    # ---------------- gold score on host (fp64) ----------------
    f64 = feats.astype(np.float64)
    emit = np.take_along_axis(f64, tags64[:, :, None], axis=2)[:, :, 0]  # (B,L)
    lmask = np.arange(L)[None, :] < lens[:, None]
    emit_sum = (emit * lmask).sum()
    mid_mask = (tags64[:, 1:] != 0)
    trans_mid = (trf[tags64[:, :-1], tags64[:, 1:]] * mid_mask).sum()
    begin = trf[START, tags64[:, 0]].sum()
    end_tag = np.take_along_axis(tags64, (lens - 1)[:, None], axis=1)[:, 0]
    end = trf[end_tag, STOP].sum()
    total_gold = emit_sum + trans_mid + begin + end

    return np.asarray(total_fwd - total_gold, np.float32)
